# revision 21
# baseline (speedup 1.0000x reference)
"""Trainium2 Bass kernel for nn_CustomCrossEntropyLoss_5368709120380.

loss = -mean_b log(y[b, t_b] + 1e-8) + sum_{b,c} w[t_b ^ c] * y[b,c] / (B*N)
where t_b = argmax_c target[b,c], w[k] = 6^popcount(k) (w[0] = 0).

Key restructure vs. the butterfly baseline: the XOR-popcount exponent is
built as a TensorEngine matmul over exact power-of-two values:

    pc(t_b ^ c) = sum_k (bit_k(t_b) << k) * 2^-k * (1 - 2*bit_k(c)) + pc(c)

so the per-row vector is just the raw AND masks [t & 2^k, .., 1] (11
wide, each value an exact power of two) and the fixed bf16 matrix C has
rows 2^-k * (1 - 2*bit_k(c)) and pc(c) -- all exact in bf16 -- so one
11-deep matmul per 128-row tile gives E[b,c] = pc(t_b ^ c) in PSUM.  The Scalar engine computes K = exp(E * ln6) = 6^pc (ln6 via
the activation's immediate scale, so no f32 matmul is needed), GpSimd
multiplies K*y, and a Copy activation with accum_out row-sums it.
The c==t_b term (weight 6^0=1, but w[0]=0) is removed by subtracting
ysel = y[t_b], fetched per-row with a GpSimd indirect_copy (diagonal of
a 16-wide group gather).

Per 128-row tile: Vector does only argmax (max + max_index) and tiny
bit ops; Tensor does transpose + 2 bf16 matmuls; Scalar does Exp and
Copy+accum (one act table, no reloads); GpSimd does the elementwise
multiply and the 2 gather ops.  All engines sit below the DMA roofline.

Sharding: pure data parallel over batch across 8 NeuronCores; each core
returns partial sums (pt_sum, ce_sum); host combines.

Self-contained: hardcodes B=65536, N=1024, 8 cores.
"""
import math

import numpy as np

import concourse.bacc as bacc
import concourse.bass as bass
import concourse.mybir as mybir
import concourse.tile as tile
from concourse.bass_utils import run_bass_kernel_spmd

F32 = mybir.dt.float32
BF16 = mybir.dt.bfloat16
U16 = mybir.dt.uint16
AX = mybir.AxisListType
OP = mybir.AluOpType
ACT = mybir.ActivationFunctionType

B_FULL = 65536
N = 1024
DIM = 10
N_CORES = 8
B_SHARD = B_FULL // N_CORES          # 8192
N_TILES = B_SHARD // 128             # 64
LN6 = math.log(6.0)

_cache = {}


def _build_program():
    nc = bacc.Bacc("TRN2", target_bir_lowering=False, debug=False)
    y_d = nc.dram_tensor("y_true", [B_SHARD, N], F32, kind="ExternalInput")
    t_d = nc.dram_tensor("target", [B_SHARD, N], F32, kind="ExternalInput")
    pw_d = nc.dram_tensor("c_pow2", [128, DIM], U16, kind="ExternalInput")
    cb_d = nc.dram_tensor("c_bits", [DIM + 1, N], BF16, kind="ExternalInput")
    id_d = nc.dram_tensor("c_ident", [128, 128], F32, kind="ExternalInput")
    cm_d = nc.dram_tensor("c_misc", [128, 48], F32, kind="ExternalInput")
    out_d = nc.dram_tensor("out", [1, 2], F32, kind="ExternalOutput")

    with tile.TileContext(nc) as tc:
        with (
            tc.tile_pool(name="const", bufs=1) as cpool,
            tc.tile_pool(name="io", bufs=4) as iopool,
            tc.tile_pool(name="lny", bufs=3) as lpool,
            tc.tile_pool(name="small", bufs=4) as spool,
            tc.tile_pool(name="scr", bufs=1) as scrpool,
            tc.tile_pool(name="strip", bufs=1) as stpool,
            tc.tile_pool(name="psE", bufs=2, space=bass.MemorySpace.PSUM) as pse,
            tc.tile_pool(name="psT", bufs=2, space=bass.MemorySpace.PSUM) as pst,
            tc.tile_pool(name="psO", bufs=1, space=bass.MemorySpace.PSUM) as pso,
        ):
            pw = cpool.tile([128, DIM], U16)
            nc.sync.dma_start(pw[:], pw_d[:])
            cb = cpool.tile([DIM + 1, N], BF16)
            nc.sync.dma_start(cb[:], cb_d[:])
            ident = cpool.tile([128, 128], F32)
            nc.sync.dma_start(ident[:], id_d[:])
            misc = cpool.tile([128, 48], F32)
            nc.sync.dma_start(misc[:], cm_d[:])
            pow2 = pw[:, 0:DIM]          # 1 << k
            diag16 = misc[:, 0:16]       # diag16[p, i] = (i == p % 16)
            zeros10 = misc[:, 16:26]     # 0.0
            zero1 = misc[:, 26:27]       # 0.0 (Exp bias)
            eps1 = misc[:, 27:28]        # 1e-8 (Ln bias)
            ones1 = misc[:, 28:29]       # 1.0 (final matmul lhsT)

            pt_strip = stpool.tile([128, N_TILES], F32)
            ys_strip = stpool.tile([128, N_TILES], F32)

            exp_q = []  # (E_psum, col) pending Exp, software-pipelined
            mul_q = []  # (K, ty, col) pending GpSimd multiply
            acc_q = []  # (Z, col) pending Copy+accum row-sum

            def drain(min_exp, min_mul, min_acc):
                while len(exp_q) > min_exp:
                    ep, j = exp_q.pop(0)
                    kt = lpool.tile([128, N], F32, tag="k")
                    nc.scalar.activation(
                        kt[:], ep[:], ACT.Exp, bias=zero1, scale=LN6
                    )
                    mul_q.append((kt, ty_ring[j % 4], j))
                while len(mul_q) > min_mul:
                    kt, tyj, j = mul_q.pop(0)
                    zt = lpool.tile([128, N], F32, tag="z")
                    nc.gpsimd.tensor_tensor(zt[:], kt[:], tyj[:], OP.mult)
                    acc_q.append((zt, j))
                while len(acc_q) > min_acc:
                    zt, j = acc_q.pop(0)
                    scrE = scrpool.tile([128, N], F32, tag="scrE")
                    nc.scalar.activation(
                        scrE[:], zt[:], ACT.Copy, bias=0.0, scale=1.0,
                        accum_out=pt_strip[:, j:j + 1],
                    )

            ty_ring = {}

            for i in range(N_TILES):
                ty = iopool.tile([128, N], F32, tag="y")
                nc.sync.dma_start(ty[:], y_d[i * 128:(i + 1) * 128, :])
                ty_ring[i % 4] = ty
                tt = iopool.tile([128, N], F32, tag="t")
                nc.scalar.dma_start(tt[:], t_d[i * 128:(i + 1) * 128, :])

                # t_p = argmax_c target[p, c] (first index on ties)
                vmax8 = spool.tile([128, 8], F32, tag="vmax8")
                nc.vector.max(vmax8[:], tt[:])
                idx = spool.tile([128, 8], U16, tag="idx")
                nc.vector.max_index(idx[:], vmax8[:], tt[:])

                # v[:, k<10] = t & (1<<k) (exact powers of two), v[:, 10] = 1
                bits_u = spool.tile([128, DIM], U16, tag="bits_u")
                nc.vector.tensor_tensor(
                    bits_u[:], pow2, idx[:, 0:1].to_broadcast((128, DIM)),
                    OP.bitwise_and,
                )
                vtile = spool.tile([128, DIM + 1], F32, tag="vtile")
                nc.vector.tensor_copy(vtile[:, 0:DIM], bits_u[:])
                nc.vector.tensor_copy(vtile[:, DIM:DIM + 1], ones1)

                # vT = vtile^T via TensorE, staged through PSUM, cast to bf16
                vT_ps = pst.tile([DIM + 1, 128], F32)
                nc.tensor.transpose(vT_ps[:], vtile[:], ident[:])
                vT = spool.tile([DIM + 1, 128], BF16, tag="vT")
                nc.vector.tensor_copy(vT[:], vT_ps[:])

                # E[b, c] = pc(t_b ^ c), two bf16 matmuls
                e_ps = pse.tile([128, N], F32)
                nc.tensor.matmul(
                    e_ps[:, 0:512], vT[:], cb[:, 0:512], start=True, stop=True
                )
                nc.tensor.matmul(
                    e_ps[:, 512:N], vT[:], cb[:, 512:N], start=True, stop=True
                )
                exp_q.append((e_ps, i))

                # ysel[p] = y[p, t_p]: group-gather 16 then diagonal
                g16 = spool.tile([128, 16], F32, tag="g16")
                nc.gpsimd.indirect_copy(g16[:], ty[:], idx[:, 0:1], True)
                scr16 = scrpool.tile([128, 16], F32, tag="scr16")
                nc.vector.scalar_tensor_tensor(
                    scr16[:], g16[:], 1.0, diag16, OP.mult, OP.mult,
                    accum_out=ys_strip[:, i:i + 1],
                )

                # K = 6^E on Scalar, Z = K*y on GpSimd, row-sum via Copy+accum
                # on Scalar -- each stage one tile behind the previous so no
                # engine ever stalls on the current tile's producers.
                drain(1, 1, 1)

            drain(0, 0, 0)

            # epilogue: pt_row -= ysel;  ce = sum ln(ysel + 1e-8)
            ly_strip = stpool.tile([128, N_TILES], F32)
            nc.scalar.activation(
                ly_strip[:], ys_strip[:], ACT.Ln, bias=eps1, scale=1.0
            )
            nc.vector.tensor_tensor(
                pt_strip[:], pt_strip[:], ys_strip[:], OP.subtract
            )

            ptsum = spool.tile([128, 1], F32, tag="ptsum")
            nc.vector.reduce_sum(ptsum[:], pt_strip[:], axis=AX.X)
            cesum = spool.tile([128, 1], F32, tag="cesum")
            nc.vector.reduce_sum(cesum[:], ly_strip[:], axis=AX.X)
            packed = spool.tile([128, 2], F32, tag="packed")
            nc.vector.tensor_copy(packed[:, 0:1], ptsum[:])
            nc.vector.tensor_copy(packed[:, 1:2], cesum[:])

            acc = pso.tile([1, 2], F32)
            nc.tensor.matmul(acc[:], ones1, packed[:], start=True, stop=True)
            sb_out = spool.tile([1, 2], F32, tag="sbout")
            nc.vector.tensor_copy(sb_out[:], acc[:])
            nc.sync.dma_start(out_d[:], sb_out[:])

    nc.compile()
    return nc


def _consts():
    import ml_dtypes

    pw = np.zeros((128, DIM), dtype=np.uint16)
    pw[:] = (1 << np.arange(DIM, dtype=np.uint16))[None, :]
    cbits = np.zeros((DIM + 1, N), dtype=np.float32)
    c = np.arange(N, dtype=np.uint32)
    pc = np.zeros(N, dtype=np.float32)
    for k in range(DIM):
        bit = ((c >> k) & 1).astype(np.float32)
        cbits[k, :] = (2.0 ** -k) * (1.0 - 2.0 * bit)
        pc += bit
    cbits[DIM, :] = pc
    cbits = cbits.astype(ml_dtypes.bfloat16)
    ident = np.eye(128, dtype=np.float32)
    misc = np.zeros((128, 48), dtype=np.float32)
    for p in range(128):
        misc[p, p % 16] = 1.0
    misc[:, 16:26] = 0.0
    misc[:, 26] = 0.0
    misc[:, 27] = 1e-8
    misc[:, 28] = 1.0
    return pw, cbits, ident, misc


def kernel(y_true: np.ndarray, target: np.ndarray) -> np.ndarray:
    assert y_true.shape == (B_FULL, N) and target.shape == (B_FULL, N)
    if "nc" not in _cache:
        _cache["nc"] = _build_program()
    nc = _cache["nc"]

    pw, cbits, ident, misc = _consts()
    in_maps = []
    for c in range(N_CORES):
        sl = slice(c * B_SHARD, (c + 1) * B_SHARD)
        in_maps.append({
            "y_true": np.ascontiguousarray(y_true[sl]),
            "target": np.ascontiguousarray(target[sl]),
            "c_pow2": pw,
            "c_bits": cbits,
            "c_ident": ident,
            "c_misc": misc,
        })

    res = run_bass_kernel_spmd(nc, in_maps, core_ids=list(range(N_CORES)))
    _cache["last_results"] = res

    pt_sum = 0.0
    ce_sum = 0.0
    for c in range(N_CORES):
        o = res.results[c]["out"]
        pt_sum += float(o[0, 0])
        ce_sum += float(o[0, 1])
    loss = -ce_sum / B_FULL + pt_sum / (B_FULL * N)
    return np.float32(loss)


# revision 25
# speedup vs baseline: 1.1059x; 1.1059x over previous
"""Trainium2 Bass kernel for nn_CustomCrossEntropyLoss_5368709120380.

loss = -mean_b log(y[b, t_b] + 1e-8) + sum_{b,c} w[t_b ^ c] * y[b,c] / (B*N)
where t_b = argmax_c target[b,c], w[k] = 6^popcount(k) (w[0] = 0).

Key restructure vs. the butterfly baseline: the XOR-popcount exponent is
built as a TensorEngine matmul over exact power-of-two values:

    pc(t_b ^ c) = sum_k (bit_k(t_b) << k) * 2^-k * (1 - 2*bit_k(c)) + pc(c)

so the per-row vector is just the raw AND masks [t & 2^k, .., 1] (11
wide, each value an exact power of two) and the fixed bf16 matrix C has
rows 2^-k * (1 - 2*bit_k(c)) and pc(c) -- all exact in bf16 -- so one
11-deep matmul per 128-row tile gives E[b,c] = pc(t_b ^ c) in PSUM.  The Scalar engine computes K = exp(E * ln6) = 6^pc (ln6 via
the activation's immediate scale, so no f32 matmul is needed), GpSimd
multiplies K*y, and a Copy activation with accum_out row-sums it.
The c==t_b term (weight 6^0=1, but w[0]=0) is removed by subtracting
ysel = y[t_b], fetched per-row with a GpSimd indirect_copy (diagonal of
a 16-wide group gather).

Per 128-row tile: Vector does only argmax (max + max_index) and tiny
bit ops; Tensor does transpose + 2 bf16 matmuls; Scalar does Exp and
Copy+accum (one act table, no reloads); GpSimd does the elementwise
multiply and the 2 gather ops.  All engines sit below the DMA roofline.

Sharding: pure data parallel over batch across 8 NeuronCores; each core
returns partial sums (pt_sum, ce_sum); host combines.

Self-contained: hardcodes B=65536, N=1024, 8 cores.
"""
import math

import numpy as np

import concourse.bacc as bacc
import concourse.bass as bass
import concourse.mybir as mybir
import concourse.tile as tile
from concourse.bass_utils import run_bass_kernel_spmd

F32 = mybir.dt.float32
BF16 = mybir.dt.bfloat16
U16 = mybir.dt.uint16
AX = mybir.AxisListType
OP = mybir.AluOpType
ACT = mybir.ActivationFunctionType

B_FULL = 65536
N = 1024
DIM = 10
N_CORES = 8
B_SHARD = B_FULL // N_CORES          # 8192
N_TILES = B_SHARD // 128             # 64
LN6 = math.log(6.0)

_cache = {}


def _build_program():
    nc = bacc.Bacc("TRN2", target_bir_lowering=False, debug=False)
    y_d = nc.dram_tensor("y_true", [B_SHARD, N], F32, kind="ExternalInput")
    t_d = nc.dram_tensor("target", [B_SHARD, N], F32, kind="ExternalInput")
    pw_d = nc.dram_tensor("c_pow2", [128, DIM], U16, kind="ExternalInput")
    cb_d = nc.dram_tensor("c_bits", [DIM + 1, N], BF16, kind="ExternalInput")
    id_d = nc.dram_tensor("c_ident", [128, 128], F32, kind="ExternalInput")
    cm_d = nc.dram_tensor("c_misc", [128, 48], F32, kind="ExternalInput")
    out_d = nc.dram_tensor("out", [1, 2], F32, kind="ExternalOutput")

    with tile.TileContext(nc) as tc:
        with (
            tc.tile_pool(name="const", bufs=1) as cpool,
            tc.tile_pool(name="io", bufs=4) as iopool,
            tc.tile_pool(name="lny", bufs=3) as lpool,
            tc.tile_pool(name="small", bufs=4) as spool,
            tc.tile_pool(name="scr", bufs=1) as scrpool,
            tc.tile_pool(name="strip", bufs=1) as stpool,
            tc.tile_pool(name="psE", bufs=2, space=bass.MemorySpace.PSUM) as pse,
            tc.tile_pool(name="psT", bufs=2, space=bass.MemorySpace.PSUM) as pst,
            tc.tile_pool(name="psO", bufs=1, space=bass.MemorySpace.PSUM) as pso,
        ):
            pw = cpool.tile([128, DIM], U16)
            nc.sync.dma_start(pw[:], pw_d[:])
            cb = cpool.tile([DIM + 1, N], BF16)
            nc.sync.dma_start(cb[:], cb_d[:])
            ident = cpool.tile([128, 128], F32)
            nc.sync.dma_start(ident[:], id_d[:])
            misc = cpool.tile([128, 48], F32)
            nc.sync.dma_start(misc[:], cm_d[:])
            pow2 = pw[:, 0:DIM]          # 1 << k
            diag16 = misc[:, 0:16]       # diag16[p, i] = (i == p % 16)
            zeros10 = misc[:, 16:26]     # 0.0
            zero1 = misc[:, 26:27]       # 0.0 (Exp bias)
            eps1 = misc[:, 27:28]        # 1e-8 (Ln bias)
            ones1 = misc[:, 28:29]       # 1.0 (final matmul lhsT)

            pt_strip = stpool.tile([128, N_TILES], F32)
            ys_strip = stpool.tile([128, N_TILES], F32)

            exp_q = []  # (E_psum, col) pending Exp, software-pipelined
            mul_q = []  # (K, ty, col) pending GpSimd multiply
            acc_q = []  # (Z, col) pending Copy+accum row-sum

            def drain(min_exp, min_mul, min_acc):
                while len(exp_q) > min_exp:
                    ep, j = exp_q.pop(0)
                    kt = lpool.tile([128, N], F32, tag="k")
                    nc.scalar.activation(
                        kt[:], ep[:], ACT.Exp, bias=zero1, scale=LN6
                    )
                    mul_q.append((kt, ty_ring[j % 4], j))
                while len(mul_q) > min_mul:
                    kt, tyj, j = mul_q.pop(0)
                    zt = lpool.tile([128, N], F32, tag="z")
                    nc.gpsimd.tensor_tensor(zt[:], kt[:], tyj[:], OP.mult)
                    acc_q.append((zt, j))
                while len(acc_q) > min_acc:
                    zt, j = acc_q.pop(0)
                    scrE = scrpool.tile([128, N], F32, tag="scrE")
                    nc.scalar.activation(
                        scrE[:], zt[:], ACT.Copy, bias=0.0, scale=1.0,
                        accum_out=pt_strip[:, j:j + 1],
                    )

            ty_ring = {}
            g16_prev = None

            for i in range(N_TILES):
                ty = iopool.tile([128, N], F32, tag="y")
                nc.sync.dma_start(ty[:], y_d[i * 128:(i + 1) * 128, :])
                ty_ring[i % 4] = ty
                tt = iopool.tile([128, N], F32, tag="t")
                nc.sync.dma_start(tt[:], t_d[i * 128:(i + 1) * 128, :])

                # t_p = argmax_c target[p, c] (first index on ties)
                vmax8 = spool.tile([128, 8], F32, tag="vmax8")
                nc.vector.max(vmax8[:], tt[:])
                idx = spool.tile([128, 8], U16, tag="idx")
                nc.vector.max_index(idx[:], vmax8[:], tt[:])

                # v[:, k<10] = t & (1<<k) (exact powers of two), v[:, 10] = 1
                bits_u = spool.tile([128, DIM], U16, tag="bits_u")
                nc.vector.tensor_tensor(
                    bits_u[:], pow2, idx[:, 0:1].to_broadcast((128, DIM)),
                    OP.bitwise_and,
                )
                vtile = spool.tile([128, DIM + 1], F32, tag="vtile")
                nc.vector.tensor_copy(vtile[:, 0:DIM], bits_u[:])
                nc.vector.tensor_copy(vtile[:, DIM:DIM + 1], ones1)

                # vT = vtile^T via TensorE, staged through PSUM, cast to bf16
                vT_ps = pst.tile([DIM + 1, 128], F32)
                nc.tensor.transpose(vT_ps[:], vtile[:], ident[:])
                vT = spool.tile([DIM + 1, 128], BF16, tag="vT")
                nc.vector.tensor_copy(vT[:], vT_ps[:])

                # E[b, c] = pc(t_b ^ c), two bf16 matmuls
                e_ps = pse.tile([128, N], F32)
                nc.tensor.matmul(
                    e_ps[:, 0:512], vT[:], cb[:, 0:512], start=True, stop=True
                )
                nc.tensor.matmul(
                    e_ps[:, 512:N], vT[:], cb[:, 512:N], start=True, stop=True
                )
                exp_q.append((e_ps, i))

                # ysel[p] = y[p, t_p]: group-gather 16 then diagonal (the
                # diagonal extract lags one tile so Vector never waits on
                # the GpSimd gather)
                g16 = spool.tile([128, 16], F32, tag="g16")
                nc.gpsimd.indirect_copy(g16[:], ty[:], idx[:, 0:1], True)
                if g16_prev is not None:
                    gp, j = g16_prev
                    scr16 = scrpool.tile([128, 16], F32, tag="scr16")
                    nc.vector.scalar_tensor_tensor(
                        scr16[:], gp[:], 1.0, diag16, OP.mult, OP.mult,
                        accum_out=ys_strip[:, j:j + 1],
                    )
                g16_prev = (g16, i)

                # K = 6^E on Scalar, Z = K*y on GpSimd, row-sum via Copy+accum
                # on Scalar -- each stage one tile behind the previous so no
                # engine ever stalls on the current tile's producers.
                drain(1, 1, 1)

            gp, j = g16_prev
            scr16 = scrpool.tile([128, 16], F32, tag="scr16")
            nc.vector.scalar_tensor_tensor(
                scr16[:], gp[:], 1.0, diag16, OP.mult, OP.mult,
                accum_out=ys_strip[:, j:j + 1],
            )
            drain(0, 0, 0)

            # epilogue: pt_row -= ysel;  ce = sum ln(ysel + 1e-8)
            ly_strip = stpool.tile([128, N_TILES], F32)
            nc.scalar.activation(
                ly_strip[:], ys_strip[:], ACT.Ln, bias=eps1, scale=1.0
            )
            nc.vector.tensor_tensor(
                pt_strip[:], pt_strip[:], ys_strip[:], OP.subtract
            )

            ptsum = spool.tile([128, 1], F32, tag="ptsum")
            nc.vector.reduce_sum(ptsum[:], pt_strip[:], axis=AX.X)
            cesum = spool.tile([128, 1], F32, tag="cesum")
            nc.vector.reduce_sum(cesum[:], ly_strip[:], axis=AX.X)
            packed = spool.tile([128, 2], F32, tag="packed")
            nc.vector.tensor_copy(packed[:, 0:1], ptsum[:])
            nc.vector.tensor_copy(packed[:, 1:2], cesum[:])

            acc = pso.tile([1, 2], F32)
            nc.tensor.matmul(acc[:], ones1, packed[:], start=True, stop=True)
            sb_out = spool.tile([1, 2], F32, tag="sbout")
            nc.vector.tensor_copy(sb_out[:], acc[:])
            nc.sync.dma_start(out_d[:], sb_out[:])

    nc.compile()
    return nc


def _consts():
    import ml_dtypes

    pw = np.zeros((128, DIM), dtype=np.uint16)
    pw[:] = (1 << np.arange(DIM, dtype=np.uint16))[None, :]
    cbits = np.zeros((DIM + 1, N), dtype=np.float32)
    c = np.arange(N, dtype=np.uint32)
    pc = np.zeros(N, dtype=np.float32)
    for k in range(DIM):
        bit = ((c >> k) & 1).astype(np.float32)
        cbits[k, :] = (2.0 ** -k) * (1.0 - 2.0 * bit)
        pc += bit
    cbits[DIM, :] = pc
    cbits = cbits.astype(ml_dtypes.bfloat16)
    ident = np.eye(128, dtype=np.float32)
    misc = np.zeros((128, 48), dtype=np.float32)
    for p in range(128):
        misc[p, p % 16] = 1.0
    misc[:, 16:26] = 0.0
    misc[:, 26] = 0.0
    misc[:, 27] = 1e-8
    misc[:, 28] = 1.0
    return pw, cbits, ident, misc


def kernel(y_true: np.ndarray, target: np.ndarray) -> np.ndarray:
    assert y_true.shape == (B_FULL, N) and target.shape == (B_FULL, N)
    if "nc" not in _cache:
        _cache["nc"] = _build_program()
    nc = _cache["nc"]

    pw, cbits, ident, misc = _consts()
    in_maps = []
    for c in range(N_CORES):
        sl = slice(c * B_SHARD, (c + 1) * B_SHARD)
        in_maps.append({
            "y_true": np.ascontiguousarray(y_true[sl]),
            "target": np.ascontiguousarray(target[sl]),
            "c_pow2": pw,
            "c_bits": cbits,
            "c_ident": ident,
            "c_misc": misc,
        })

    res = run_bass_kernel_spmd(nc, in_maps, core_ids=list(range(N_CORES)))
    _cache["last_results"] = res

    pt_sum = 0.0
    ce_sum = 0.0
    for c in range(N_CORES):
        o = res.results[c]["out"]
        pt_sum += float(o[0, 0])
        ce_sum += float(o[0, 1])
    loss = -ce_sum / B_FULL + pt_sum / (B_FULL * N)
    return np.float32(loss)


# revision 27
# speedup vs baseline: 1.1128x; 1.0062x over previous
"""Trainium2 Bass kernel for nn_CustomCrossEntropyLoss_5368709120380.

loss = -mean_b log(y[b, t_b] + 1e-8) + sum_{b,c} w[t_b ^ c] * y[b,c] / (B*N)
where t_b = argmax_c target[b,c], w[k] = 6^popcount(k) (w[0] = 0).

Key restructure vs. the butterfly baseline: the XOR-popcount exponent is
built as a TensorEngine matmul over exact power-of-two values:

    pc(t_b ^ c) = sum_k (bit_k(t_b) << k) * 2^-k * (1 - 2*bit_k(c)) + pc(c)

so the per-row vector is just the raw AND masks [t & 2^k, .., 1] (11
wide, each value an exact power of two) and the fixed bf16 matrix C has
rows 2^-k * (1 - 2*bit_k(c)) and pc(c) -- all exact in bf16 -- so one
11-deep matmul per 128-row tile gives E[b,c] = pc(t_b ^ c) in PSUM.  The Scalar engine computes K = exp(E * ln6) = 6^pc (ln6 via
the activation's immediate scale, so no f32 matmul is needed), GpSimd
multiplies K*y, and a Copy activation with accum_out row-sums it.
The c==t_b term (weight 6^0=1, but w[0]=0) is removed by subtracting
ysel = y[t_b], fetched per-row with a GpSimd indirect_copy (diagonal of
a 16-wide group gather).

Per 128-row tile: Vector does only argmax (max + max_index) and tiny
bit ops; Tensor does transpose + 2 bf16 matmuls; Scalar does Exp and
Copy+accum (one act table, no reloads); GpSimd does the elementwise
multiply and the 2 gather ops.  All engines sit below the DMA roofline.

Sharding: pure data parallel over batch across 8 NeuronCores; each core
returns partial sums (pt_sum, ce_sum); host combines.

Self-contained: hardcodes B=65536, N=1024, 8 cores.
"""
import math

import numpy as np

import concourse.bacc as bacc
import concourse.bass as bass
import concourse.mybir as mybir
import concourse.tile as tile
from concourse.bass_utils import run_bass_kernel_spmd

F32 = mybir.dt.float32
BF16 = mybir.dt.bfloat16
U16 = mybir.dt.uint16
AX = mybir.AxisListType
OP = mybir.AluOpType
ACT = mybir.ActivationFunctionType

B_FULL = 65536
N = 1024
DIM = 10
N_CORES = 8
B_SHARD = B_FULL // N_CORES          # 8192
N_TILES = B_SHARD // 128             # 64
LN6 = math.log(6.0)

_cache = {}


def _build_program():
    nc = bacc.Bacc("TRN2", target_bir_lowering=False, debug=False)
    y_d = nc.dram_tensor("y_true", [B_SHARD, N], F32, kind="ExternalInput")
    t_d = nc.dram_tensor("target", [B_SHARD, N], F32, kind="ExternalInput")
    pw_d = nc.dram_tensor("c_pow2", [128, DIM], U16, kind="ExternalInput")
    cb_d = nc.dram_tensor("c_bits", [DIM + 1, N], BF16, kind="ExternalInput")
    id_d = nc.dram_tensor("c_ident", [128, 128], F32, kind="ExternalInput")
    cm_d = nc.dram_tensor("c_misc", [128, 48], F32, kind="ExternalInput")
    out_d = nc.dram_tensor("out", [1, 2], F32, kind="ExternalOutput")

    with tile.TileContext(nc) as tc:
        with (
            tc.tile_pool(name="const", bufs=1) as cpool,
            tc.tile_pool(name="io", bufs=6) as iopool,
            tc.tile_pool(name="lny", bufs=3) as lpool,
            tc.tile_pool(name="small", bufs=4) as spool,
            tc.tile_pool(name="scr", bufs=1) as scrpool,
            tc.tile_pool(name="strip", bufs=1) as stpool,
            tc.tile_pool(name="psE", bufs=2, space=bass.MemorySpace.PSUM) as pse,
            tc.tile_pool(name="psT", bufs=2, space=bass.MemorySpace.PSUM) as pst,
            tc.tile_pool(name="psO", bufs=1, space=bass.MemorySpace.PSUM) as pso,
        ):
            pw = cpool.tile([128, DIM], U16)
            nc.sync.dma_start(pw[:], pw_d[:])
            cb = cpool.tile([DIM + 1, N], BF16)
            nc.sync.dma_start(cb[:], cb_d[:])
            ident = cpool.tile([128, 128], F32)
            nc.sync.dma_start(ident[:], id_d[:])
            misc = cpool.tile([128, 48], F32)
            nc.sync.dma_start(misc[:], cm_d[:])
            pow2 = pw[:, 0:DIM]          # 1 << k
            diag16 = misc[:, 0:16]       # diag16[p, i] = (i == p % 16)
            zeros10 = misc[:, 16:26]     # 0.0
            zero1 = misc[:, 26:27]       # 0.0 (Exp bias)
            eps1 = misc[:, 27:28]        # 1e-8 (Ln bias)
            ones1 = misc[:, 28:29]       # 1.0 (final matmul lhsT)

            pt_strip = stpool.tile([128, N_TILES], F32)
            ys_strip = stpool.tile([128, N_TILES], F32)

            exp_q = []  # (E_psum, col) pending Exp, software-pipelined
            mul_q = []  # (K, ty, col) pending GpSimd multiply
            acc_q = []  # (Z, col) pending Copy+accum row-sum

            def drain(min_exp, min_mul, min_acc):
                while len(exp_q) > min_exp:
                    ep, j = exp_q.pop(0)
                    kt = lpool.tile([128, N], F32, tag="k")
                    nc.scalar.activation(
                        kt[:], ep[:], ACT.Exp, bias=zero1, scale=LN6
                    )
                    mul_q.append((kt, ty_ring[j % 4], j))
                while len(mul_q) > min_mul:
                    kt, tyj, j = mul_q.pop(0)
                    zt = lpool.tile([128, N], F32, tag="z")
                    nc.gpsimd.tensor_tensor(zt[:], kt[:], tyj[:], OP.mult)
                    acc_q.append((zt, j))
                while len(acc_q) > min_acc:
                    zt, j = acc_q.pop(0)
                    scrE = scrpool.tile([128, N], F32, tag="scrE")
                    nc.scalar.activation(
                        scrE[:], zt[:], ACT.Copy, bias=0.0, scale=1.0,
                        accum_out=pt_strip[:, j:j + 1],
                    )

            ty_ring = {}
            g16_prev = None

            for i in range(N_TILES):
                ty = iopool.tile([128, N], F32, tag="y")
                nc.sync.dma_start(ty[:], y_d[i * 128:(i + 1) * 128, :])
                ty_ring[i % 4] = ty
                tt = iopool.tile([128, N], F32, tag="t")
                nc.sync.dma_start(tt[:], t_d[i * 128:(i + 1) * 128, :])

                # t_p = argmax_c target[p, c] (first index on ties)
                vmax8 = spool.tile([128, 8], F32, tag="vmax8")
                nc.vector.max(vmax8[:], tt[:])
                idx = spool.tile([128, 8], U16, tag="idx")
                nc.vector.max_index(idx[:], vmax8[:], tt[:])

                # v[:, k<10] = t & (1<<k) (exact powers of two), v[:, 10] = 1
                bits_u = spool.tile([128, DIM], U16, tag="bits_u")
                nc.vector.tensor_scalar(
                    bits_u[:], pow2, idx[:, 0:1], None, OP.bitwise_and
                )
                vtile = spool.tile([128, DIM + 1], F32, tag="vtile")
                nc.vector.tensor_copy(vtile[:, 0:DIM], bits_u[:])
                nc.vector.tensor_copy(vtile[:, DIM:DIM + 1], ones1)

                # vT = vtile^T via TensorE, staged through PSUM, cast to bf16
                vT_ps = pst.tile([DIM + 1, 128], F32)
                nc.tensor.transpose(vT_ps[:], vtile[:], ident[:])
                vT = spool.tile([DIM + 1, 128], BF16, tag="vT")
                nc.scalar.activation(vT[:], vT_ps[:], ACT.Copy, bias=0.0, scale=1.0)

                # E[b, c] = pc(t_b ^ c), two bf16 matmuls
                e_ps = pse.tile([128, N], F32)
                nc.tensor.matmul(
                    e_ps[:, 0:512], vT[:], cb[:, 0:512], start=True, stop=True
                )
                nc.tensor.matmul(
                    e_ps[:, 512:N], vT[:], cb[:, 512:N], start=True, stop=True
                )
                exp_q.append((e_ps, i))

                # ysel[p] = y[p, t_p]: group-gather 16 then diagonal (the
                # diagonal extract lags one tile so Vector never waits on
                # the GpSimd gather)
                g16 = spool.tile([128, 16], F32, tag="g16")
                nc.gpsimd.indirect_copy(g16[:], ty[:], idx[:, 0:1], True)
                if g16_prev is not None:
                    gp, j = g16_prev
                    scr16 = scrpool.tile([128, 16], F32, tag="scr16")
                    nc.vector.scalar_tensor_tensor(
                        scr16[:], gp[:], 1.0, diag16, OP.mult, OP.mult,
                        accum_out=ys_strip[:, j:j + 1],
                    )
                g16_prev = (g16, i)

                # K = 6^E on Scalar, Z = K*y on GpSimd, row-sum via Copy+accum
                # on Scalar -- each stage one tile behind the previous so no
                # engine ever stalls on the current tile's producers.
                drain(1, 1, 1)

            gp, j = g16_prev
            scr16 = scrpool.tile([128, 16], F32, tag="scr16")
            nc.vector.scalar_tensor_tensor(
                scr16[:], gp[:], 1.0, diag16, OP.mult, OP.mult,
                accum_out=ys_strip[:, j:j + 1],
            )
            drain(0, 0, 0)

            # epilogue: pt_row -= ysel;  ce = sum ln(ysel + 1e-8)
            ly_strip = stpool.tile([128, N_TILES], F32)
            nc.scalar.activation(
                ly_strip[:], ys_strip[:], ACT.Ln, bias=eps1, scale=1.0
            )
            nc.vector.tensor_tensor(
                pt_strip[:], pt_strip[:], ys_strip[:], OP.subtract
            )

            ptsum = spool.tile([128, 1], F32, tag="ptsum")
            nc.vector.reduce_sum(ptsum[:], pt_strip[:], axis=AX.X)
            cesum = spool.tile([128, 1], F32, tag="cesum")
            nc.vector.reduce_sum(cesum[:], ly_strip[:], axis=AX.X)
            packed = spool.tile([128, 2], F32, tag="packed")
            nc.vector.tensor_copy(packed[:, 0:1], ptsum[:])
            nc.vector.tensor_copy(packed[:, 1:2], cesum[:])

            acc = pso.tile([1, 2], F32)
            nc.tensor.matmul(acc[:], ones1, packed[:], start=True, stop=True)
            sb_out = spool.tile([1, 2], F32, tag="sbout")
            nc.vector.tensor_copy(sb_out[:], acc[:])
            nc.sync.dma_start(out_d[:], sb_out[:])

    nc.compile()
    return nc


def _consts():
    import ml_dtypes

    pw = np.zeros((128, DIM), dtype=np.uint16)
    pw[:] = (1 << np.arange(DIM, dtype=np.uint16))[None, :]
    cbits = np.zeros((DIM + 1, N), dtype=np.float32)
    c = np.arange(N, dtype=np.uint32)
    pc = np.zeros(N, dtype=np.float32)
    for k in range(DIM):
        bit = ((c >> k) & 1).astype(np.float32)
        cbits[k, :] = (2.0 ** -k) * (1.0 - 2.0 * bit)
        pc += bit
    cbits[DIM, :] = pc
    cbits = cbits.astype(ml_dtypes.bfloat16)
    ident = np.eye(128, dtype=np.float32)
    misc = np.zeros((128, 48), dtype=np.float32)
    for p in range(128):
        misc[p, p % 16] = 1.0
    misc[:, 16:26] = 0.0
    misc[:, 26] = 0.0
    misc[:, 27] = 1e-8
    misc[:, 28] = 1.0
    return pw, cbits, ident, misc


def kernel(y_true: np.ndarray, target: np.ndarray) -> np.ndarray:
    assert y_true.shape == (B_FULL, N) and target.shape == (B_FULL, N)
    if "nc" not in _cache:
        _cache["nc"] = _build_program()
    nc = _cache["nc"]

    pw, cbits, ident, misc = _consts()
    in_maps = []
    for c in range(N_CORES):
        sl = slice(c * B_SHARD, (c + 1) * B_SHARD)
        in_maps.append({
            "y_true": np.ascontiguousarray(y_true[sl]),
            "target": np.ascontiguousarray(target[sl]),
            "c_pow2": pw,
            "c_bits": cbits,
            "c_ident": ident,
            "c_misc": misc,
        })

    res = run_bass_kernel_spmd(nc, in_maps, core_ids=list(range(N_CORES)))
    _cache["last_results"] = res

    pt_sum = 0.0
    ce_sum = 0.0
    for c in range(N_CORES):
        o = res.results[c]["out"]
        pt_sum += float(o[0, 0])
        ce_sum += float(o[0, 1])
    loss = -ce_sum / B_FULL + pt_sum / (B_FULL * N)
    return np.float32(loss)
